# revision 5
# baseline (speedup 1.0000x reference)
"""v2: scores phase restructured to dt-outer/chunk-inner (lhsT pair reuse +
PSUM bank interleave — the measured-fastest PE pattern), PSUM rebalanced.

Also: dup_* flags re-emit a phase's PE work into scratch so the marginal HW
cost of each phase can be measured with the slope method (timing only;
results unchanged).

See kernel.py for the algorithm description (column-softmax causal
attention, 4 batches x 2 key-parity shards).
"""
import numpy as np
import ml_dtypes
from contextlib import ExitStack

import concourse.bass as bass
import concourse.tile as tile
import concourse.bacc as bacc
import concourse.mybir as mybir
from concourse.bass_utils import run_bass_kernel_spmd

B, N, D = 4, 2048, 1024
NT = N // 128
ET = D // 128
G = 8
SCALE = 1.0 / np.sqrt(D).astype(np.float32)
NEGBIG = -1.0e9

BF = mybir.dt.bfloat16
F32 = mybir.dt.float32

WOFF = []
_o = 0
for _g in range(G):
    WOFF.append(_o)
    _o += N - 256 * _g
WTOT = _o  # 9216


def _score_chunks(g):
    """(q0, width) chunks covering [256g, N); first chunk holds the 256
    masked columns; widths 512/256."""
    width = N - 256 * g
    q0 = 256 * g
    chunks = []
    if (width // 256) % 2 == 1:
        chunks.append((q0, 256))
        q0 += 256
    while q0 < N:
        chunks.append((q0, 512))
        q0 += 512
    return chunks


def _emit_body(nc, tc, pools, aps, dup=(), scores_mode="pair", preloaded=None):
    (xtpool, xkpool, ypool, vpool, vppool, zpool, stpool,
     ps_yv, ps_s, ps_av, scratch) = pools
    (xT_d, xkT_d, a2t, wvt, mb_sb, wt_sb, out_d) = aps

    if preloaded is not None:
        xk, xt = preloaded
    else:
        xk = []
        for t in range(ET):
            xktile = xkpool.tile([128, D], BF, tag="xk")
            # trigger input DMAs from ACT: its stream finishes a body ~7us
            # before SP's (whose last out-trigger waits on the av(7) stage
            # copy), so the next body's loads start earlier
            nc.scalar.dma_start(xktile[:], xkT_d[t * 128:(t + 1) * 128, :])
            xk.append(xktile)
        xt = []
        for t in range(ET):
            xtile = xtpool.tile([128, N], BF, tag="xt")
            nc.scalar.dma_start(xtile[:], xT_d[t * 128:(t + 1) * 128, :])
            xt.append(xtile)

    def emit_y(dst_tiles):
        yts = []
        for dt_ in range(ET):
            ytile = dst_tiles()
            pt = [ps_yv.tile([128, 512], F32, tag="psyv", name=f"psy{c}")
                  for c in range(2)]
            for e in range(ET):
                for c in range(2):
                    nc.tensor.matmul(
                        pt[c][:],
                        a2t[e][:, dt_ * 128:(dt_ + 1) * 128],
                        xk[e][:, c * 512:(c + 1) * 512],
                        start=(e == 0), stop=(e == ET - 1),
                    )
            for c in range(2):
                nc.vector.tensor_copy(ytile[:, c * 512:(c + 1) * 512], pt[c][:])
            yts.append(ytile)
        return yts

    yt = emit_y(lambda: ypool.tile([128, D], BF, tag="yt", name="yt"))
    if "y" in dup:
        for dt_ in range(ET):
            pt = [ps_yv.tile([128, 512], F32, tag="psyv", name=f"dy{c}")
                  for c in range(2)]
            for e in range(ET):
                for c in range(2):
                    nc.tensor.matmul(
                        pt[c][:], a2t[e][:, dt_ * 128:(dt_ + 1) * 128],
                        xk[e][:, c * 512:(c + 1) * 512],
                        start=(e == 0), stop=(e == ET - 1))
    if "y3" in dup:
        for rep in range(3):
            for dt_ in range(ET):
                pt = [ps_yv.tile([128, 512], F32, tag="psyv", name=f"d3_{c}")
                      for c in range(2)]
                for e in range(ET):
                    for c in range(2):
                        nc.tensor.matmul(
                            pt[c][:], a2t[e][:, dt_ * 128:(dt_ + 1) * 128],
                            xk[e][:, c * 512:(c + 1) * 512],
                            start=(e == 0), stop=(e == ET - 1))
    if "yone" in dup:
        # microbench-like in situ: rhs from ONE tile, lhsT alternates two tiles
        for dt_ in range(ET):
            pt = [ps_yv.tile([128, 512], F32, tag="psyv", name=f"do{c}")
                  for c in range(2)]
            for e in range(ET):
                for c in range(2):
                    nc.tensor.matmul(
                        pt[c][:], a2t[e % 2][:, dt_ * 128:(dt_ + 1) * 128],
                        xk[0][:, c * 512:(c + 1) * 512],
                        start=(e == 0), stop=(e == ET - 1))
    if "yrev" in dup:
        # e-inner: lhsT changes EVERY matmul; each c-chunk is an 8-chain
        # into a single bank (accum8 pattern)
        for dt_ in range(ET):
            pt = [ps_yv.tile([128, 512], F32, tag="psyv", name=f"dr{c}")
                  for c in range(2)]
            for c in range(2):
                for e in range(ET):
                    nc.tensor.matmul(
                        pt[c][:], a2t[e][:, dt_ * 128:(dt_ + 1) * 128],
                        xk[e][:, c * 512:(c + 1) * 512],
                        start=(e == 0), stop=(e == ET - 1))

    def emit_v(dst_tiles):
        vts = []
        for g in range(G):
            vtile = dst_tiles()
            pt = [ps_yv.tile([128, 512], F32, tag="psyv", name=f"psv{c}")
                  for c in range(2)]
            for e in range(ET):
                for c in range(2):
                    nc.tensor.matmul(
                        pt[c][:],
                        xk[e][:, g * 128:(g + 1) * 128],
                        wvt[e][:, c * 512:(c + 1) * 512],
                        start=(e == 0), stop=(e == ET - 1),
                    )
            for c in range(2):
                nc.vector.tensor_copy(vtile[:, c * 512:(c + 1) * 512], pt[c][:])
            vts.append(vtile)
        return vts

    vt = emit_v(lambda: vpool.tile([128, D], BF, tag="v", name="vt"))
    if "v" in dup:
        for g in range(G):
            pt = [ps_yv.tile([128, 512], F32, tag="psyv", name=f"dv{c}")
                  for c in range(2)]
            for e in range(ET):
                for c in range(2):
                    nc.tensor.matmul(
                        pt[c][:], xk[e][:, g * 128:(g + 1) * 128],
                        wvt[e][:, c * 512:(c + 1) * 512],
                        start=(e == 0), stop=(e == ET - 1))

    vp = [None] * G

    def emit_av(g, dest_wt=None, dest_out=None):
        for qt in (2 * g, 2 * g + 1):
            stage = stpool.tile([128, D], BF, tag="st", name="st")
            apt = [ps_av.tile([128, 512], F32, tag="psav", name=f"psav{c}")
                   for c in range(2)]
            for gg in range(g + 1):
                lhs = wt_sb[:, WOFF[gg] + 128 * qt - 256 * gg:
                               WOFF[gg] + 128 * qt - 256 * gg + 128]
                for oc in range(2):
                    nc.tensor.matmul(apt[oc][:], lhs,
                                     vp[gg][:, oc * 512:(oc + 1) * 512],
                                     start=(gg == 0), stop=(gg == g))
            # split the drain across DVE and ACT so short early chains
            # aren't gated by two serial DVE copies
            nc.vector.tensor_copy(stage[:, 0:512], apt[0][:])
            nc.scalar.copy(stage[:, 512:1024], apt[1][:])
            nc.sync.dma_start(out_d[qt * 128:(qt + 1) * 128, :], stage[:])

    def emit_av_dup(g):
        for qt in (2 * g, 2 * g + 1):
            apt = [ps_av.tile([128, 512], F32, tag="psav", name=f"dav{c}")
                   for c in range(2)]
            for gg in range(g + 1):
                lhs = wt_sb[:, WOFF[gg] + 128 * qt - 256 * gg:
                               WOFF[gg] + 128 * qt - 256 * gg + 128]
                for oc in range(2):
                    nc.tensor.matmul(apt[oc][:], lhs,
                                     vp[gg][:, oc * 512:(oc + 1) * 512],
                                     start=(gg == 0), stop=(gg == g))

    def _score_drain(g, wt_dst, zp, ci, q0, w, pt):
        """mask-add (first chunk) + exp into wt + column sum into zp."""
        if ci == 0:
            nc.vector.tensor_add(pt[:, :256], pt[:, :256], mb_sb[:])
        nc.scalar.activation(
            wt_dst[:, WOFF[g] + (q0 - 256 * g): WOFF[g] + (q0 - 256 * g) + w],
            pt[:, :w],
            mybir.ActivationFunctionType.Exp,
            scale=float(SCALE),
            accum_out=zp[:, ci:ci + 1],
        )

    def emit_scores(g, wt_dst, zp, mode="pair"):
        chunks = _score_chunks(g)
        if mode == "base":
            # baseline: chunk-serial, each chunk an 8-deep chain in one bank
            for ci, (q0, w) in enumerate(chunks):
                pt = ps_s.tile([128, 512], F32, tag="pss", name="pss")
                for dt_ in range(ET):
                    nc.tensor.matmul(
                        pt[:, :w], yt[dt_][:, g * 128:(g + 1) * 128],
                        xt[dt_][:, q0:q0 + w],
                        start=(dt_ == 0), stop=(dt_ == ET - 1),
                    )
                _score_drain(g, wt_dst, zp, ci, q0, w, pt)
            return len(chunks)
        # pair mode: chunks processed two at a time with their dt-chains
        # interleaved -- lhsT loaded once per dt (pair reuse), PSUM banks
        # alternate between the two chunks (il2pair, fastest measured).
        ci = 0
        while ci < len(chunks):
            pair = chunks[ci:ci + 2]
            pts = [ps_s.tile([128, 512], F32, tag="pss", name="pss")
                   for _ in pair]
            for dt_ in range(ET):
                lhs = yt[dt_][:, g * 128:(g + 1) * 128]
                for pj, (q0, w) in enumerate(pair):
                    nc.tensor.matmul(
                        pts[pj][:, :w], lhs, xt[dt_][:, q0:q0 + w],
                        start=(dt_ == 0), stop=(dt_ == ET - 1),
                    )
            for pj, (q0, w) in enumerate(pair):
                _score_drain(g, wt_dst, zp, ci + pj, q0, w, pts[pj])
            ci += 2
        return len(chunks)

    for g in range(G):
        zp = zpool.tile([128, 4], F32, tag="zp")
        nch = emit_scores(g, wt_sb, zp, mode=scores_mode)
        if "s" in dup:
            chunks_d = _score_chunks(g)
            ci_d = 0
            while ci_d < len(chunks_d):
                pair_d = chunks_d[ci_d:ci_d + 2]
                pts_d = [ps_s.tile([128, 512], F32, tag="pss", name="dps")
                         for _ in pair_d]
                for dt_ in range(ET):
                    lhs = yt[dt_][:, g * 128:(g + 1) * 128]
                    for pj, (q0, w) in enumerate(pair_d):
                        nc.tensor.matmul(
                            pts_d[pj][:, :w], lhs, xt[dt_][:, q0:q0 + w],
                            start=(dt_ == 0), stop=(dt_ == ET - 1))
                ci_d += 2
        if nch == 1:
            rz = zpool.tile([128, 1], F32, tag="rz")
            nc.vector.reciprocal(rz[:], zp[:, 0:1])
        else:
            z = zpool.tile([128, 1], F32, tag="z")
            nc.vector.tensor_reduce(z[:], zp[:, :nch],
                                    axis=mybir.AxisListType.X,
                                    op=mybir.AluOpType.add)
            rz = zpool.tile([128, 1], F32, tag="rz")
            nc.vector.reciprocal(rz[:], z[:])
        vptile = vppool.tile([128, D], BF, tag="vp")
        nc.scalar.mul(vptile[:], vt[g][:], rz[:])
        vp[g] = vptile
    # AV block after ALL score groups: every vp[g] is ready long before its
    # chain ends, so the exp->Z->1/Z->mul latency never stalls PE (it could
    # not hide under the short late-group score blocks when interleaved)
    for g in reversed(range(G)):
        emit_av(g)
        if "av" in dup:
            emit_av_dup(g)


def build_program(with_loop=False, max_iters=2048, unroll=4, dup=(),
                  scores_mode="pair", no_dma=False):
    nc = bacc.Bacc("TRN2", target_bir_lowering=False, debug=False, num_devices=8)
    xT_d = nc.dram_tensor("xT", [D, N], BF, kind="ExternalInput").ap()
    xkT_d = nc.dram_tensor("xkT", [D, D], BF, kind="ExternalInput").ap()
    a2_d = nc.dram_tensor("a2", [D, D], BF, kind="ExternalInput").ap()
    wvT_d = nc.dram_tensor("wvT", [D, D], BF, kind="ExternalInput").ap()
    mb_d = nc.dram_tensor("maskbias", [128, 256], F32, kind="ExternalInput").ap()
    out_d = nc.dram_tensor("out", [N, D], BF, kind="ExternalOutput").ap()
    if with_loop:
        n_d = nc.dram_tensor("niter", [1, 1], mybir.dt.int32,
                             kind="ExternalInput").ap()

    with tile.TileContext(nc) as tc:
        with ExitStack() as ctx:
            persist = ctx.enter_context(tc.tile_pool(name="persist", bufs=1))
            xtpool = ctx.enter_context(tc.tile_pool(name="xt", bufs=2 * ET))
            xkpool = ctx.enter_context(tc.tile_pool(name="xk", bufs=ET))
            ypool = ctx.enter_context(tc.tile_pool(name="yT", bufs=ET))
            vpool = ctx.enter_context(tc.tile_pool(name="v", bufs=G))
            vppool = ctx.enter_context(tc.tile_pool(name="vp", bufs=G))
            zpool = ctx.enter_context(tc.tile_pool(name="z", bufs=4 * G))
            stpool = ctx.enter_context(tc.tile_pool(name="stage", bufs=4))
            ps_yv = ctx.enter_context(tc.tile_pool(name="ps_yv", bufs=2,
                                                   space="PSUM"))
            ps_s = ctx.enter_context(tc.tile_pool(name="ps_s", bufs=4,
                                                  space="PSUM"))
            ps_av = ctx.enter_context(tc.tile_pool(name="ps_av", bufs=2,
                                                   space="PSUM"))

            mb_sb = persist.tile([128, 256], F32, tag="mb")
            nc.sync.dma_start(mb_sb[:], mb_d[:])
            a2t = []
            for t in range(ET):
                w = persist.tile([128, D], BF, tag=f"a2_{t}")
                nc.sync.dma_start(w[:], a2_d[t * 128:(t + 1) * 128, :])
                a2t.append(w)
            wvt = []
            for t in range(ET):
                w = persist.tile([128, D], BF, tag=f"wv_{t}")
                nc.sync.dma_start(w[:], wvT_d[t * 128:(t + 1) * 128, :])
                wvt.append(w)
            wt_sb = persist.tile([128, WTOT], BF, tag="wt")
            scratch = None
            preloaded = None
            if no_dma:
                pxk, pxt = [], []
                for t in range(ET):
                    w = persist.tile([128, D], BF, tag=f"pxk_{t}")
                    nc.sync.dma_start(w[:], xkT_d[t * 128:(t + 1) * 128, :])
                    pxk.append(w)
                for t in range(ET):
                    w = persist.tile([128, N], BF, tag=f"pxt_{t}")
                    nc.sync.dma_start(w[:], xT_d[t * 128:(t + 1) * 128, :])
                    pxt.append(w)
                preloaded = (pxk, pxt)

            pools = (xtpool, xkpool, ypool, vpool, vppool, zpool, stpool,
                     ps_yv, ps_s, ps_av, scratch)
            aps = (xT_d, xkT_d, a2t, wvt, mb_sb, wt_sb, out_d)

            if with_loop:
                n_sb = persist.tile([1, 1], mybir.dt.int32, tag="niter")
                nc.sync.dma_start(n_sb[:], n_d[:])
                regs = []
                with tc.tile_critical():
                    for e, eng in nc.engines.items():
                        r = eng.alloc_register(f"niter_{e.name}")
                        eng.reg_load(r, n_sb[0:1, 0:1])
                        regs.append(r)
                n_val = nc.snap(bass.RegisterHandles(regs), min_val=0,
                                max_val=max_iters)
                with tc.For_i(0, n_val, 1, staggered_reset=True,
                              hint_engines=(mybir.EngineType.PE,)):
                    for _ in range(unroll):
                        _emit_body(nc, tc, pools, aps, dup=dup,
                                   scores_mode=scores_mode,
                                   preloaded=preloaded)
            else:
                _emit_body(nc, tc, pools, aps, dup=dup,
                           scores_mode=scores_mode, preloaded=preloaded)

    nc.compile()
    return nc


def prepare_in_maps(x, Wq, Wk, Wv, niter=None):
    x = np.asarray(x, dtype=np.float32)
    A2 = (np.asarray(Wk, np.float32).T @ np.asarray(Wq, np.float32))
    a2_bf = A2.astype(ml_dtypes.bfloat16)
    wvT_bf = np.asarray(Wv, np.float32).T.astype(ml_dtypes.bfloat16)
    mb = []
    for h in range(2):
        i = np.arange(128)[:, None]
        j = np.arange(256)[None, :]
        mb.append(np.where(j >= 2 * i + h, 0.0, NEGBIG).astype(np.float32))
    in_maps = []
    for c in range(8):
        b, h = c // 2, c % 2
        xTb = x[b].T.astype(ml_dtypes.bfloat16)
        m = {
            "xT": xTb,
            "xkT": np.ascontiguousarray(xTb[:, h::2]),
            "a2": a2_bf,
            "wvT": wvT_bf,
            "maskbias": mb[h],
        }
        if niter is not None:
            m["niter"] = np.array([[niter]], dtype=np.int32)
        in_maps.append(m)
    return in_maps


def gather_out(results):
    out = np.empty((B, N, D), np.float32)
    for b in range(B):
        out[b] = (results[2 * b]["out"].astype(np.float32)
                  + results[2 * b + 1]["out"].astype(np.float32))
    return out


_CACHE = {}


def kernel(x, Wq, Wk, Wv):
    if "nc" not in _CACHE:
        _CACHE["nc"] = build_program(with_loop=False)
    nc = _CACHE["nc"]
    in_maps = prepare_in_maps(x, Wq, Wk, Wv)
    res = run_bass_kernel_spmd(nc, in_maps, list(range(8)), trace=False)
    return gather_out(res.results)


# revision 6
# speedup vs baseline: 1.0120x; 1.0120x over previous
"""v2: scores phase restructured to dt-outer/chunk-inner (lhsT pair reuse +
PSUM bank interleave — the measured-fastest PE pattern), PSUM rebalanced.

Also: dup_* flags re-emit a phase's PE work into scratch so the marginal HW
cost of each phase can be measured with the slope method (timing only;
results unchanged).

See kernel.py for the algorithm description (column-softmax causal
attention, 4 batches x 2 key-parity shards).
"""
import numpy as np
import ml_dtypes
from contextlib import ExitStack

import concourse.bass as bass
import concourse.tile as tile
import concourse.bacc as bacc
import concourse.mybir as mybir
from concourse.bass_utils import run_bass_kernel_spmd

B, N, D = 4, 2048, 1024
NT = N // 128
ET = D // 128
G = 8
SCALE = 1.0 / np.sqrt(D).astype(np.float32)
NEGBIG = -1.0e9

BF = mybir.dt.bfloat16
F32 = mybir.dt.float32

WOFF = []
_o = 0
for _g in range(G):
    WOFF.append(_o)
    _o += N - 256 * _g
WTOT = _o  # 9216


def _score_chunks(g):
    """(q0, width) chunks covering [256g, N); first chunk holds the 256
    masked columns; widths 512/256."""
    width = N - 256 * g
    q0 = 256 * g
    chunks = []
    if (width // 256) % 2 == 1:
        chunks.append((q0, 256))
        q0 += 256
    while q0 < N:
        chunks.append((q0, 512))
        q0 += 512
    return chunks


def _emit_body(nc, tc, pools, aps, dup=(), scores_mode="pair", preloaded=None):
    (xtpool, xkpool, ypool, vpool, vppool, zpool, stpool,
     ps_yv, ps_s, ps_av, scratch) = pools
    (xT_d, xkT_d, a2t, wvt, mb_sb, wt_sb, out_d) = aps

    if preloaded is not None:
        xk, xt = preloaded
    else:
        xk = []
        for t in range(ET):
            xktile = xkpool.tile([128, D], BF, tag="xk")
            # trigger input DMAs from ACT: its stream finishes a body ~7us
            # before SP's (whose last out-trigger waits on the av(7) stage
            # copy), so the next body's loads start earlier
            nc.scalar.dma_start(xktile[:], xkT_d[t * 128:(t + 1) * 128, :])
            xk.append(xktile)
        xt = []
        for t in range(ET):
            xtile = xtpool.tile([128, N], BF, tag="xt")
            nc.scalar.dma_start(xtile[:], xT_d[t * 128:(t + 1) * 128, :])
            xt.append(xtile)

    def emit_y(dst_tiles):
        yts = []
        for dt_ in range(ET):
            ytile = dst_tiles()
            pt = [ps_yv.tile([128, 512], F32, tag="psyv", name=f"psy{c}")
                  for c in range(2)]
            for e in range(ET):
                for c in range(2):
                    nc.tensor.matmul(
                        pt[c][:],
                        a2t[e][:, dt_ * 128:(dt_ + 1) * 128],
                        xk[e][:, c * 512:(c + 1) * 512],
                        start=(e == 0), stop=(e == ET - 1),
                    )
            for c in range(2):
                nc.vector.tensor_copy(ytile[:, c * 512:(c + 1) * 512], pt[c][:])
            yts.append(ytile)
        return yts

    yt = emit_y(lambda: ypool.tile([128, D], BF, tag="yt", name="yt"))
    if "y" in dup:
        for dt_ in range(ET):
            pt = [ps_yv.tile([128, 512], F32, tag="psyv", name=f"dy{c}")
                  for c in range(2)]
            for e in range(ET):
                for c in range(2):
                    nc.tensor.matmul(
                        pt[c][:], a2t[e][:, dt_ * 128:(dt_ + 1) * 128],
                        xk[e][:, c * 512:(c + 1) * 512],
                        start=(e == 0), stop=(e == ET - 1))
    if "y3" in dup:
        for rep in range(3):
            for dt_ in range(ET):
                pt = [ps_yv.tile([128, 512], F32, tag="psyv", name=f"d3_{c}")
                      for c in range(2)]
                for e in range(ET):
                    for c in range(2):
                        nc.tensor.matmul(
                            pt[c][:], a2t[e][:, dt_ * 128:(dt_ + 1) * 128],
                            xk[e][:, c * 512:(c + 1) * 512],
                            start=(e == 0), stop=(e == ET - 1))
    if "yone" in dup:
        # microbench-like in situ: rhs from ONE tile, lhsT alternates two tiles
        for dt_ in range(ET):
            pt = [ps_yv.tile([128, 512], F32, tag="psyv", name=f"do{c}")
                  for c in range(2)]
            for e in range(ET):
                for c in range(2):
                    nc.tensor.matmul(
                        pt[c][:], a2t[e % 2][:, dt_ * 128:(dt_ + 1) * 128],
                        xk[0][:, c * 512:(c + 1) * 512],
                        start=(e == 0), stop=(e == ET - 1))
    if "yrev" in dup:
        # e-inner: lhsT changes EVERY matmul; each c-chunk is an 8-chain
        # into a single bank (accum8 pattern)
        for dt_ in range(ET):
            pt = [ps_yv.tile([128, 512], F32, tag="psyv", name=f"dr{c}")
                  for c in range(2)]
            for c in range(2):
                for e in range(ET):
                    nc.tensor.matmul(
                        pt[c][:], a2t[e][:, dt_ * 128:(dt_ + 1) * 128],
                        xk[e][:, c * 512:(c + 1) * 512],
                        start=(e == 0), stop=(e == ET - 1))

    def emit_v(dst_tiles):
        vts = []
        for g in range(G):
            vtile = dst_tiles()
            pt = [ps_yv.tile([128, 512], F32, tag="psyv", name=f"psv{c}")
                  for c in range(2)]
            for e in range(ET):
                for c in range(2):
                    nc.tensor.matmul(
                        pt[c][:],
                        xk[e][:, g * 128:(g + 1) * 128],
                        wvt[e][:, c * 512:(c + 1) * 512],
                        start=(e == 0), stop=(e == ET - 1),
                    )
            for c in range(2):
                nc.vector.tensor_copy(vtile[:, c * 512:(c + 1) * 512], pt[c][:])
            vts.append(vtile)
        return vts

    vt = emit_v(lambda: vpool.tile([128, D], BF, tag="v", name="vt"))
    if "v" in dup:
        for g in range(G):
            pt = [ps_yv.tile([128, 512], F32, tag="psyv", name=f"dv{c}")
                  for c in range(2)]
            for e in range(ET):
                for c in range(2):
                    nc.tensor.matmul(
                        pt[c][:], xk[e][:, g * 128:(g + 1) * 128],
                        wvt[e][:, c * 512:(c + 1) * 512],
                        start=(e == 0), stop=(e == ET - 1))

    vp = [None] * G

    def emit_av(g, dest_wt=None, dest_out=None):
        for qt in (2 * g, 2 * g + 1):
            stage = stpool.tile([128, D], BF, tag="st", name="st")
            apt = [ps_av.tile([128, 512], F32, tag="psav", name=f"psav{c}")
                   for c in range(2)]
            for gg in range(g + 1):
                lhs = wt_sb[:, WOFF[gg] + 128 * qt - 256 * gg:
                               WOFF[gg] + 128 * qt - 256 * gg + 128]
                for oc in range(2):
                    nc.tensor.matmul(apt[oc][:], lhs,
                                     vp[gg][:, oc * 512:(oc + 1) * 512],
                                     start=(gg == 0), stop=(gg == g))
            # split the drain across DVE and ACT so short early chains
            # aren't gated by two serial DVE copies
            nc.vector.tensor_copy(stage[:, 0:512], apt[0][:])
            nc.scalar.copy(stage[:, 512:1024], apt[1][:])
            nc.sync.dma_start(out_d[qt * 128:(qt + 1) * 128, :], stage[:])

    def emit_av_dup(g):
        for qt in (2 * g, 2 * g + 1):
            apt = [ps_av.tile([128, 512], F32, tag="psav", name=f"dav{c}")
                   for c in range(2)]
            for gg in range(g + 1):
                lhs = wt_sb[:, WOFF[gg] + 128 * qt - 256 * gg:
                               WOFF[gg] + 128 * qt - 256 * gg + 128]
                for oc in range(2):
                    nc.tensor.matmul(apt[oc][:], lhs,
                                     vp[gg][:, oc * 512:(oc + 1) * 512],
                                     start=(gg == 0), stop=(gg == g))

    def _score_drain(g, wt_dst, zp, ci, q0, w, pt):
        """mask-add (first chunk) + exp into wt + column sum into zp."""
        if ci == 0:
            nc.vector.tensor_add(pt[:, :256], pt[:, :256], mb_sb[:])
        nc.scalar.activation(
            wt_dst[:, WOFF[g] + (q0 - 256 * g): WOFF[g] + (q0 - 256 * g) + w],
            pt[:, :w],
            mybir.ActivationFunctionType.Exp,
            scale=float(SCALE),
            accum_out=zp[:, ci:ci + 1],
        )

    def emit_scores(g, wt_dst, zp, mode="pair"):
        chunks = _score_chunks(g)
        if mode == "base":
            # baseline: chunk-serial, each chunk an 8-deep chain in one bank
            for ci, (q0, w) in enumerate(chunks):
                pt = ps_s.tile([128, 512], F32, tag="pss", name="pss")
                for dt_ in range(ET):
                    nc.tensor.matmul(
                        pt[:, :w], yt[dt_][:, g * 128:(g + 1) * 128],
                        xt[dt_][:, q0:q0 + w],
                        start=(dt_ == 0), stop=(dt_ == ET - 1),
                    )
                _score_drain(g, wt_dst, zp, ci, q0, w, pt)
            return len(chunks)
        # pair mode: chunks processed two at a time with their dt-chains
        # interleaved -- lhsT loaded once per dt (pair reuse), PSUM banks
        # alternate between the two chunks (il2pair, fastest measured).
        ci = 0
        while ci < len(chunks):
            pair = chunks[ci:ci + 2]
            pts = [ps_s.tile([128, 512], F32, tag="pss", name="pss")
                   for _ in pair]
            for dt_ in range(ET):
                lhs = yt[dt_][:, g * 128:(g + 1) * 128]
                for pj, (q0, w) in enumerate(pair):
                    nc.tensor.matmul(
                        pts[pj][:, :w], lhs, xt[dt_][:, q0:q0 + w],
                        start=(dt_ == 0), stop=(dt_ == ET - 1),
                    )
            for pj, (q0, w) in enumerate(pair):
                _score_drain(g, wt_dst, zp, ci + pj, q0, w, pts[pj])
            ci += 2
        return len(chunks)

    for g in range(G):
        zp = zpool.tile([128, 4], F32, tag="zp")
        nch = emit_scores(g, wt_sb, zp, mode=scores_mode)
        if "s" in dup:
            chunks_d = _score_chunks(g)
            ci_d = 0
            while ci_d < len(chunks_d):
                pair_d = chunks_d[ci_d:ci_d + 2]
                pts_d = [ps_s.tile([128, 512], F32, tag="pss", name="dps")
                         for _ in pair_d]
                for dt_ in range(ET):
                    lhs = yt[dt_][:, g * 128:(g + 1) * 128]
                    for pj, (q0, w) in enumerate(pair_d):
                        nc.tensor.matmul(
                            pts_d[pj][:, :w], lhs, xt[dt_][:, q0:q0 + w],
                            start=(dt_ == 0), stop=(dt_ == ET - 1))
                ci_d += 2
        if nch == 1:
            rz = zpool.tile([128, 1], F32, tag="rz")
            nc.vector.reciprocal(rz[:], zp[:, 0:1])
        else:
            z = zpool.tile([128, 1], F32, tag="z")
            nc.vector.tensor_reduce(z[:], zp[:, :nch],
                                    axis=mybir.AxisListType.X,
                                    op=mybir.AluOpType.add)
            rz = zpool.tile([128, 1], F32, tag="rz")
            nc.vector.reciprocal(rz[:], z[:])
        vptile = vppool.tile([128, D], BF, tag="vp")
        nc.scalar.mul(vptile[:], vt[g][:], rz[:])
        vp[g] = vptile
        if g > 0:
            emit_av(g - 1)
            if "av" in dup:
                emit_av_dup(g - 1)
    emit_av(G - 1)
    if "av" in dup:
        emit_av_dup(G - 1)



def build_program(with_loop=False, max_iters=2048, unroll=4, dup=(),
                  scores_mode="pair", no_dma=False):
    nc = bacc.Bacc("TRN2", target_bir_lowering=False, debug=False, num_devices=8)
    xT_d = nc.dram_tensor("xT", [D, N], BF, kind="ExternalInput").ap()
    xkT_d = nc.dram_tensor("xkT", [D, D], BF, kind="ExternalInput").ap()
    a2_d = nc.dram_tensor("a2", [D, D], BF, kind="ExternalInput").ap()
    wvT_d = nc.dram_tensor("wvT", [D, D], BF, kind="ExternalInput").ap()
    mb_d = nc.dram_tensor("maskbias", [128, 256], F32, kind="ExternalInput").ap()
    out_d = nc.dram_tensor("out", [N, D], BF, kind="ExternalOutput").ap()
    if with_loop:
        n_d = nc.dram_tensor("niter", [1, 1], mybir.dt.int32,
                             kind="ExternalInput").ap()

    with tile.TileContext(nc) as tc:
        with ExitStack() as ctx:
            persist = ctx.enter_context(tc.tile_pool(name="persist", bufs=1))
            xtpool = ctx.enter_context(tc.tile_pool(name="xt", bufs=2 * ET))
            xkpool = ctx.enter_context(tc.tile_pool(name="xk", bufs=ET))
            ypool = ctx.enter_context(tc.tile_pool(name="yT", bufs=ET))
            vpool = ctx.enter_context(tc.tile_pool(name="v", bufs=G))
            vppool = ctx.enter_context(tc.tile_pool(name="vp", bufs=G))
            zpool = ctx.enter_context(tc.tile_pool(name="z", bufs=4 * G))
            stpool = ctx.enter_context(tc.tile_pool(name="stage", bufs=4))
            ps_yv = ctx.enter_context(tc.tile_pool(name="ps_yv", bufs=2,
                                                   space="PSUM"))
            ps_s = ctx.enter_context(tc.tile_pool(name="ps_s", bufs=4,
                                                  space="PSUM"))
            ps_av = ctx.enter_context(tc.tile_pool(name="ps_av", bufs=2,
                                                   space="PSUM"))

            mb_sb = persist.tile([128, 256], F32, tag="mb")
            nc.sync.dma_start(mb_sb[:], mb_d[:])
            a2t = []
            for t in range(ET):
                w = persist.tile([128, D], BF, tag=f"a2_{t}")
                nc.sync.dma_start(w[:], a2_d[t * 128:(t + 1) * 128, :])
                a2t.append(w)
            wvt = []
            for t in range(ET):
                w = persist.tile([128, D], BF, tag=f"wv_{t}")
                nc.sync.dma_start(w[:], wvT_d[t * 128:(t + 1) * 128, :])
                wvt.append(w)
            wt_sb = persist.tile([128, WTOT], BF, tag="wt")
            scratch = None
            preloaded = None
            if no_dma:
                pxk, pxt = [], []
                for t in range(ET):
                    w = persist.tile([128, D], BF, tag=f"pxk_{t}")
                    nc.sync.dma_start(w[:], xkT_d[t * 128:(t + 1) * 128, :])
                    pxk.append(w)
                for t in range(ET):
                    w = persist.tile([128, N], BF, tag=f"pxt_{t}")
                    nc.sync.dma_start(w[:], xT_d[t * 128:(t + 1) * 128, :])
                    pxt.append(w)
                preloaded = (pxk, pxt)

            pools = (xtpool, xkpool, ypool, vpool, vppool, zpool, stpool,
                     ps_yv, ps_s, ps_av, scratch)
            aps = (xT_d, xkT_d, a2t, wvt, mb_sb, wt_sb, out_d)

            if with_loop:
                n_sb = persist.tile([1, 1], mybir.dt.int32, tag="niter")
                nc.sync.dma_start(n_sb[:], n_d[:])
                regs = []
                with tc.tile_critical():
                    for e, eng in nc.engines.items():
                        r = eng.alloc_register(f"niter_{e.name}")
                        eng.reg_load(r, n_sb[0:1, 0:1])
                        regs.append(r)
                n_val = nc.snap(bass.RegisterHandles(regs), min_val=0,
                                max_val=max_iters)
                with tc.For_i(0, n_val, 1, staggered_reset=True,
                              hint_engines=(mybir.EngineType.PE,)):
                    for _ in range(unroll):
                        _emit_body(nc, tc, pools, aps, dup=dup,
                                   scores_mode=scores_mode,
                                   preloaded=preloaded)
            else:
                _emit_body(nc, tc, pools, aps, dup=dup,
                           scores_mode=scores_mode, preloaded=preloaded)

    nc.compile()
    return nc


def prepare_in_maps(x, Wq, Wk, Wv, niter=None):
    x = np.asarray(x, dtype=np.float32)
    A2 = (np.asarray(Wk, np.float32).T @ np.asarray(Wq, np.float32))
    a2_bf = A2.astype(ml_dtypes.bfloat16)
    wvT_bf = np.asarray(Wv, np.float32).T.astype(ml_dtypes.bfloat16)
    mb = []
    for h in range(2):
        i = np.arange(128)[:, None]
        j = np.arange(256)[None, :]
        mb.append(np.where(j >= 2 * i + h, 0.0, NEGBIG).astype(np.float32))
    in_maps = []
    for c in range(8):
        b, h = c // 2, c % 2
        xTb = x[b].T.astype(ml_dtypes.bfloat16)
        m = {
            "xT": xTb,
            "xkT": np.ascontiguousarray(xTb[:, h::2]),
            "a2": a2_bf,
            "wvT": wvT_bf,
            "maskbias": mb[h],
        }
        if niter is not None:
            m["niter"] = np.array([[niter]], dtype=np.int32)
        in_maps.append(m)
    return in_maps


def gather_out(results):
    out = np.empty((B, N, D), np.float32)
    for b in range(B):
        out[b] = (results[2 * b]["out"].astype(np.float32)
                  + results[2 * b + 1]["out"].astype(np.float32))
    return out


_CACHE = {}


def kernel(x, Wq, Wk, Wv):
    if "nc" not in _CACHE:
        _CACHE["nc"] = build_program(with_loop=False)
    nc = _CACHE["nc"]
    in_maps = prepare_in_maps(x, Wq, Wk, Wv)
    res = run_bass_kernel_spmd(nc, in_maps, list(range(8)), trace=False)
    return gather_out(res.results)
